# revision 45
# baseline (speedup 1.0000x reference)
"""Trainium2 Bass kernel for paged causal self-attention (GQA + YaRN rope).

Sharding: tensor-parallel over heads. Core c (of 8) owns kv-head c and
q-heads 2c, 2c+1 for both batches. Each core computes a partial output
y_c = attn_c @ Wo_c.T over its 256 channels; the host sums the 8 partials.

The reference's scatter of new K/V into the pools is dead code w.r.t. the
returned output (slot_map is a permutation, so gathered past slots are
disjoint from the scattered new slots); new K/V are consumed directly from
SBUF and only the past 1024 slots per batch are gathered via indirect DMA.

All matmul operands are bf16 with fp32 PSUM (moving free dim is capped at
512 by the fp32-PSUM bank size). Diagonal-block score/AV matmuls are
narrowed to the causally visible query range. Softmax denominators use two
interleaved sum chains (DVE + GPSIMD), a ones-vector PE reduce/broadcast,
and the fast-approx DVE reciprocal. x tiles stream 4 contraction chunks
per DMA to amortize the per-DMA issue cost on the sync ring; bulk
constants load on the scalar ring. PSUM->SBUF output copies are spread
across scalar, vector, and gpsimd; output partials are written as bf16.
"""

import sys

sys.path.insert(0, "/opt/trn_rl_repo")

import ml_dtypes
import numpy as np

import concourse.bacc as bacc
import concourse.bass_isa as bass_isa
import concourse.tile as tile
from concourse import mybir
from concourse.bass import IndirectOffsetOnAxis
from concourse.bass_utils import run_bass_kernel_spmd

F32 = mybir.dt.float32
BF16 = mybir.dt.bfloat16
I32 = mybir.dt.int32
EXP = mybir.ActivationFunctionType.Exp
RADD = bass_isa.ReduceOp.add

B, T, PAST = 2, 1024, 1024
H, HKV, D = 16, 8, 128
G = H // HKV            # q heads per kv head
C = H * D               # 2048
TOTAL = PAST + T        # 2048
NB = B * T              # 2048 flattened tokens
NCORES = 8
P = 128
TB = 512                # token block
NEG = -1.0e30


def _emit(tc, io):
    nc = tc.nc
    (xT, wq, wk, wv, wo, kp, vp, gidx, cosq, sinq, cosk, sink,
     cmask, rperm, ident, ones, y) = io

    with (
        tc.tile_pool(name="const", bufs=1) as cp,
        tc.tile_pool(name="persist", bufs=1) as pp,
        tc.tile_pool(name="ysb", bufs=6) as yp,
    ):
        # ---- prologue DMAs: weights split across BOTH HWDGE rings so the
        # kc=0..3 matmuls can start as early as possible ----
        wq_t = pp.tile([P, 16, G * P], BF16)
        wk_t = pp.tile([P, 16, P], BF16)
        wv_t = pp.tile([P, 16, P], BF16)
        gidx_t = cp.tile([P, 2 * 8], I32)
        for lo, hi in ((0, 4), (4, 16)):
            nc.scalar.dma_start(wq_t[:, lo:hi, :], wq[:, lo:hi, :])
            nc.scalar.dma_start(wk_t[:, lo:hi, :], wk[:, lo:hi, :])
            nc.scalar.dma_start(wv_t[:, lo:hi, :], wv[:, lo:hi, :])
            if hi == 4:
                nc.scalar.dma_start(gidx_t[:], gidx[:])

        # past K/V gather on the gpsimd SWDGE queue (overlaps everything)
        kg = [None, None]
        vg = [None, None]
        for b in range(B):
            kg[b] = pp.tile([P, 8, P], BF16, name=f"kg{b}", tag=f"kg{b}")
            vg[b] = pp.tile([P, 8, P], BF16, name=f"vg{b}", tag=f"vg{b}")
            for j in range(8):
                # [P,1]-index gathers: the multi-column offset-AP form
                # miscomputes on hardware
                nc.gpsimd.indirect_dma_start(
                    out=kg[b][:, j, :],
                    out_offset=None,
                    in_=kp[:, :],
                    in_offset=IndirectOffsetOnAxis(
                        ap=gidx_t[:, 8 * b + j:8 * b + j + 1], axis=0),
                )
                nc.gpsimd.indirect_dma_start(
                    out=vg[b][:, j, :],
                    out_offset=None,
                    in_=vp[:, :],
                    in_offset=IndirectOffsetOnAxis(
                        ap=gidx_t[:, 8 * b + j:8 * b + j + 1], axis=0),
                )

        # ---- persistent activations ----
        qT0 = pp.tile([P, NB], BF16)      # q head 2c,   [d, token]
        qT1 = pp.tile([P, NB], BF16)      # q head 2c+1
        kT_new = pp.tile([P, NB], BF16)   # new keys,    [d, token]
        v_nat = pp.tile([P, B, 8, P], BF16)    # new values, [t%128, b, chunk, d]
        kT_past = pp.tile([P, B, 8, P], BF16)  # past keys,  [d, b, chunk, s%128]
        att0 = pp.tile([P, NB], BF16)     # attention out head 2c, [d, token]
        att1 = pp.tile([P, NB], BF16)
        wo_t = pp.tile([P, G, C], BF16)

        # ================= phase 1: projections + rope =================
        with (
            tc.tile_pool(name="xt", bufs=8) as xp,
            tc.tile_pool(name="rope", bufs=2) as rp,
            tc.tile_pool(name="pproj", bufs=1, space="PSUM") as pjp,
            tc.tile_pool(name="prope", bufs=2, space="PSUM") as rpp,
            tc.tile_pool(name="ptr", bufs=2, space="PSUM") as trp,
        ):
            cosq_t = cp.tile([P, T], BF16)
            sinq_t = cp.tile([P, T], BF16)
            cosk_t = cp.tile([P, T], BF16)
            sink_t = cp.tile([P, T], BF16)
            rperm_t = cp.tile([P, P], BF16)
            ident_t = cp.tile([P, P], BF16)
            ones_t = cp.tile([P, P], BF16)
            cmask_t = cp.tile([P, 4, TB], BF16)

            for tb in range(NB // TB):           # 4 token blocks of 512
                n0 = tb * TB
                b = tb // 2
                tpos = (tb % 2) * TB             # position-in-batch of block start

                q0p = pjp.tile([P, TB], F32, name="q0p", tag="q0")
                q1p = pjp.tile([P, TB], F32, name="q1p", tag="q1")
                kkp = pjp.tile([P, TB], F32, name="kkp", tag="kk")
                vvp = pjp.tile([P, TB], F32, name="vvp", tag="vv")
                xTr = xT.rearrange("(k p) n -> p k n", p=P)
                for q4 in range(4):              # 4 contraction chunks per DMA
                    xt = xp.tile([P, 4, TB], BF16, name="xt", tag="xt")
                    nc.sync.dma_start(
                        xt[:], xTr[:, 4 * q4:4 * q4 + 4, n0:n0 + TB])
                    for sub in range(4):
                        kc = 4 * q4 + sub
                        st = (kc == 0)
                        sp = (kc == 15)
                        x_ap = xt[:, sub, :]
                        nc.tensor.matmul(q0p[:], wq_t[:, kc, 0:P], x_ap, start=st, stop=sp)
                        nc.tensor.matmul(q1p[:], wq_t[:, kc, P:2 * P], x_ap, start=st, stop=sp)
                        nc.tensor.matmul(kkp[:], wk_t[:, kc, :], x_ap, start=st, stop=sp)
                        nc.tensor.matmul(vvp[:], wv_t[:, kc, :], x_ap, start=st, stop=sp)

                if tb == 0:
                    # rope tables + attention constants: needed only after
                    # the first projection block; emitted after the first xt
                    # stream so they don't delay the first matmuls
                    nc.scalar.dma_start(cosq_t[:], cosq[:])
                    nc.scalar.dma_start(sinq_t[:], sinq[:])
                    nc.scalar.dma_start(cosk_t[:], cosk[:])
                    nc.scalar.dma_start(sink_t[:], sink[:])
                    nc.scalar.dma_start(rperm_t[:], rperm[:])
                    nc.scalar.dma_start(ident_t[:], ident[:])
                    nc.scalar.dma_start(ones_t[:], ones[:])
                    nc.scalar.dma_start(cmask_t[:], cmask[:])
                    nc.scalar.dma_start(wo_t[:], wo[:])

                # rope for q0, q1, k
                for src, dst, ct, stt in (
                    (q0p, qT0, cosq_t, sinq_t),
                    (q1p, qT1, cosq_t, sinq_t),
                    (kkp, kT_new, cosk_t, sink_t),
                ):
                    raw = rp.tile([P, TB], BF16, name="raw", tag="raw")
                    nc.vector.tensor_copy(raw[:], src[:])
                    rot = rpp.tile([P, TB], F32, name="rot", tag="rot")
                    nc.tensor.matmul(rot[:], rperm_t[:], raw[:], start=True, stop=True)
                    dslice = dst[:, n0:n0 + TB]
                    nc.vector.tensor_mul(dslice, raw[:], ct[:, tpos:tpos + TB])
                    tmp = rp.tile([P, TB], BF16, name="tmp", tag="tmp")
                    nc.vector.tensor_mul(tmp[:], rot[:], stt[:, tpos:tpos + TB])
                    nc.vector.tensor_add(dslice, dslice, tmp[:])

                # v: no rope; transpose [d, t] -> [t, d] in 128-chunks
                vraw = rp.tile([P, TB], BF16, name="vraw", tag="vraw")
                nc.scalar.copy(vraw[:], vvp[:])
                for j4 in range(TB // P):
                    vt = trp.tile([P, P], BF16, name="vt", tag="tr")
                    nc.tensor.transpose(vt[:], vraw[:, j4 * P:(j4 + 1) * P],
                                        ident_t[:])
                    nc.vector.tensor_copy(v_nat[:, b, (tb % 2) * 4 + j4, :], vt[:])

                # past K transpose, 4 chunks per block: [s, d] -> [d, s]
                for j in range(4 * tb, 4 * tb + 4):
                    bb, jj = j // 8, j % 8
                    kt = trp.tile([P, P], BF16, name="kt", tag="tr")
                    nc.tensor.transpose(kt[:], kg[bb][:, jj, :], ident_t[:])
                    nc.vector.tensor_copy(kT_past[:, bb, jj, :], kt[:])

        # ================= phase 2: attention + output proj =================
        with (
            tc.tile_pool(name="exps", bufs=10) as ep,
            tc.tile_pool(name="sums", bufs=3) as sp_,
            tc.tile_pool(name="pscore", bufs=3, space="PSUM") as scp,
            tc.tile_pool(name="pav", bufs=2, space="PSUM") as avp,
            tc.tile_pool(name="pbc", bufs=1, space="PSUM") as bcp,
            tc.tile_pool(name="py", bufs=2, space="PSUM") as pyp,
        ):
            def attn_group(b, tbq, g):
                qT, att = ((qT0, att0), (qT1, att1))[g]
                t0 = b * T + tbq * TB
                q_ap = qT[:, t0:t0 + TB]
                njnew = 4 * tbq + 4
                nch = 8 + njnew
                sum_e = sp_.tile([P, TB], BF16, name="sum_e", tag="sum_e")
                sum_o = sp_.tile([P, TB], BF16, name="sum_o", tag="sum_o")
                av = avp.tile([P, TB], F32, name="av", tag="av")

                chunks = [(kT_past[:, b, j, :], vg[b][:, j, :], None)
                          for j in range(8)]
                for j in range(njnew):
                    koff = b * T + j * P
                    ri = j - 4 * tbq
                    chunks.append((kT_new[:, koff:koff + P],
                                   v_nat[:, b, j, :],
                                   ri if ri >= 0 else None))

                for ci, (k_ap, v_ap, mri) in enumerate(chunks):
                    # diagonal blocks: only queries >= mri*128 see this
                    # key chunk; narrow all work to that range
                    r0 = mri * P if mri else 0
                    s_ps = scp.tile([P, TB], F32, name="s_ps", tag="s")
                    nc.tensor.matmul(s_ps[:, r0:], k_ap, q_ap[:, r0:],
                                     start=True, stop=True)
                    if mri is not None:
                        nc.vector.tensor_add(s_ps[:, r0:], s_ps[:, r0:],
                                             cmask_t[:, mri, r0:])
                    e = ep.tile([P, TB], BF16, name="e", tag="e")
                    nc.scalar.activation(e[:, r0:], s_ps[:, r0:], EXP)
                    # two interleaved sum chains halve the add latency;
                    # the odd chain runs on the mostly-idle gpsimd
                    eng = nc.vector if ci % 2 == 0 else nc.gpsimd
                    tgt = sum_e if ci % 2 == 0 else sum_o
                    if ci < 2:
                        eng.tensor_copy(tgt[:], e[:])
                    else:
                        eng.tensor_add(tgt[:, r0:], tgt[:, r0:], e[:, r0:])
                    nc.tensor.matmul(av[:, r0:], v_ap, e[:, r0:],
                                     start=(ci == 0), stop=(ci == nch - 1),
                                     skip_group_check=True)

                nc.vector.tensor_add(sum_e[:], sum_e[:], sum_o[:])
                # softmax denominator: reduce over partitions + bcast
                tsum = scp.tile([1, TB], F32, name="tsum", tag="s")
                nc.tensor.matmul(tsum[:], ones_t[:, 0:1], sum_e[:],
                                 start=True, stop=True)
                rinv = sp_.tile([1, TB], F32, name="rinv", tag="rinv")
                nc.vector.reciprocal_approx_fast(rinv[:], tsum[:])
                rinvb = sp_.tile([1, TB], BF16, name="rinvb", tag="rinvb")
                nc.vector.tensor_copy(rinvb[:], rinv[:])
                rbc = bcp.tile([P, TB], F32, name="rbc", tag="rbc")
                nc.tensor.matmul(rbc[:], ones_t[0:1, :], rinvb[:],
                                 start=True, stop=True)
                rbs = sp_.tile([P, TB], BF16, name="rbs", tag="rbs")
                nc.scalar.copy(rbs[:], rbc[:])
                nc.vector.tensor_mul(att[:, t0:t0 + TB], av[:], rbs[:])

            def outproj(b, tbq):
                t0 = b * T + tbq * TB
                for tc4 in range(4):
                    tt0 = t0 + tc4 * P
                    for cb in range(4):
                        yps = pyp.tile([P, TB], F32, name="yps", tag="y")
                        nc.tensor.matmul(yps[:], att0[:, tt0:tt0 + P],
                                         wo_t[:, 0, cb * TB:(cb + 1) * TB],
                                         start=True, stop=False)
                        nc.tensor.matmul(yps[:], att1[:, tt0:tt0 + P],
                                         wo_t[:, 1, cb * TB:(cb + 1) * TB],
                                         start=False, stop=True)
                        ysb = yp.tile([P, TB], BF16, name="ysbt", tag="ysbt")
                        # balance psum->sbuf copies (gpsimd cannot read
                        # PSUM; ACT carries the exps)
                        if cb % 2 == 0:
                            nc.scalar.copy(ysb[:], yps[:])
                        else:
                            nc.vector.tensor_copy(ysb[:], yps[:])
                        nc.sync.dma_start(
                            y[tt0:tt0 + P, cb * TB:(cb + 1) * TB], ysb[:])

            # interleave the two batches' groups: adjacent groups are
            # independent, giving the scheduler deeper parallel work; each
            # batch's output projection is emitted as soon as its second
            # head finishes so it overlaps the other batch's attention
            for tbq in range(2):
                attn_group(0, tbq, 0)
                attn_group(1, tbq, 0)
                attn_group(0, tbq, 1)
                outproj(0, tbq)
                attn_group(1, tbq, 1)
                outproj(1, tbq)


def build_nc():
    nc = bacc.Bacc("TRN2")
    xT = nc.dram_tensor("xT", [C, NB], BF16, kind="ExternalInput")
    wq = nc.dram_tensor("wq", [P, 16, G * P], BF16, kind="ExternalInput")
    wk = nc.dram_tensor("wk", [P, 16, P], BF16, kind="ExternalInput")
    wv = nc.dram_tensor("wv", [P, 16, P], BF16, kind="ExternalInput")
    wo = nc.dram_tensor("wo", [P, G, C], BF16, kind="ExternalInput")
    kp = nc.dram_tensor("kp", [B * TOTAL, D], BF16, kind="ExternalInput")
    vp = nc.dram_tensor("vp", [B * TOTAL, D], BF16, kind="ExternalInput")
    gidx = nc.dram_tensor("gidx", [P, B * 8], I32, kind="ExternalInput")
    cosq = nc.dram_tensor("cosq", [P, T], BF16, kind="ExternalInput")
    sinq = nc.dram_tensor("sinq", [P, T], BF16, kind="ExternalInput")
    cosk = nc.dram_tensor("cosk", [P, T], BF16, kind="ExternalInput")
    sink = nc.dram_tensor("sink", [P, T], BF16, kind="ExternalInput")
    cmask = nc.dram_tensor("cmask", [P, 4, TB], BF16, kind="ExternalInput")
    rperm = nc.dram_tensor("rperm", [P, P], BF16, kind="ExternalInput")
    ident = nc.dram_tensor("ident", [P, P], BF16, kind="ExternalInput")
    ones = nc.dram_tensor("ones", [P, P], BF16, kind="ExternalInput")
    y = nc.dram_tensor("y", [NB, C], BF16, kind="ExternalOutput")
    io = (xT, wq, wk, wv, wo, kp, vp, gidx, cosq, sinq, cosk, sink,
          cmask, rperm, ident, ones, y)
    with nc.allow_low_precision(reason="bf16 matmul operands and outputs"):
        with tile.TileContext(nc) as tc:
            _emit(tc, io)
    nc.compile()
    return nc


def host_inputs(x, Wq, Wkv, Wo, K_pool, V_pool, slot_map, past_len):
    bf16 = ml_dtypes.bfloat16
    x = np.asarray(x, dtype=np.float32)
    Wq = np.asarray(Wq, dtype=np.float32)
    Wkv = np.asarray(Wkv, dtype=np.float32)
    Wo = np.asarray(Wo, dtype=np.float32)
    K_pool = np.asarray(K_pool, dtype=np.float32)
    V_pool = np.asarray(V_pool, dtype=np.float32)
    slot_map = np.asarray(slot_map, dtype=np.int32)
    past = int(past_len)
    assert past == PAST, f"kernel hardcodes past_len={PAST}, got {past}"

    xT = np.ascontiguousarray(x.reshape(NB, C).T.astype(bf16))

    # rope tables; argument arithmetic mirrors the f32 ops of the reference
    idx = np.arange(D // 2, dtype=np.float32)
    inv = np.float32(1.0) / np.float32(10000.0) ** (idx / np.float32(D // 2))
    inv = inv.astype(np.float32)
    t = np.arange(past, past + T, dtype=np.float32)
    freqs = (t[:, None] * inv[None, :]).astype(np.float32)
    emb = np.concatenate([freqs, freqs], axis=1)
    cos = np.cos(emb).astype(np.float32)
    sin = np.sin(emb).astype(np.float32)
    qscale = np.float32(1.0) / np.sqrt(np.float32(D))
    cosqT = np.ascontiguousarray((cos * qscale).T.astype(bf16))
    sinqT = np.ascontiguousarray((sin * qscale).T.astype(bf16))
    coskT = np.ascontiguousarray(cos.T.astype(bf16))
    sinkT = np.ascontiguousarray(sin.T.astype(bf16))

    s_i = np.arange(P, dtype=np.int64)[:, None]
    t_i = np.arange(TB, dtype=np.int64)[None, :]
    cm = np.empty((P, 4, TB), np.float32)
    for ri in range(4):
        cm[:, ri, :] = np.where(s_i <= t_i - ri * P, 0.0, NEG)
    cm = cm.astype(bf16)

    gidx = slot_map[:, :past].reshape(B, 8, P).transpose(2, 0, 1).reshape(P, B * 8)
    gidx = np.ascontiguousarray(gidx.astype(np.int32))

    rperm = np.zeros((P, P), np.float32)
    for d in range(D // 2):
        rperm[d + D // 2, d] = -1.0       # rot(q)[d] = -q[d+64] for d < 64
        rperm[d, d + D // 2] = 1.0        # rot(q)[d] = q[d-64] for d >= 64
    rperm = rperm.astype(bf16)
    ident = np.eye(P, dtype=np.float32).astype(bf16)
    ones = np.ones((P, P), np.float32).astype(bf16)

    def stat_layout(wT, nkc, m):
        # [C, m] contract-major -> [P, nkc, m] (partition-major SBUF layout)
        return np.ascontiguousarray(
            wT.reshape(nkc, P, m).transpose(1, 0, 2).astype(bf16))

    in_maps = []
    for c in range(NCORES):
        wq_c = stat_layout(Wq[G * D * c:G * D * (c + 1), :].T, 16, G * D)
        wk_c = stat_layout(Wkv[D * c:D * (c + 1), :].T, 16, D)
        wv_c = stat_layout(
            Wkv[HKV * D + D * c:HKV * D + D * (c + 1), :].T, 16, D)
        wo_c = stat_layout(Wo[:, G * D * c:G * D * (c + 1)].T, G, C)
        in_maps.append({
            "xT": xT,
            "wq": wq_c,
            "wk": wk_c,
            "wv": wv_c,
            "wo": wo_c,
            "kp": np.ascontiguousarray(K_pool[:, c, :].astype(bf16)),
            "vp": np.ascontiguousarray(V_pool[:, c, :].astype(bf16)),
            "gidx": gidx,
            "cosq": cosqT, "sinq": sinqT, "cosk": coskT, "sink": sinkT,
            "cmask": cm, "rperm": rperm, "ident": ident, "ones": ones,
        })
    return in_maps


_NC_CACHE = None


def kernel(**inputs):
    global _NC_CACHE
    in_maps = host_inputs(**inputs)
    if _NC_CACHE is None:
        _NC_CACHE = build_nc()
    res = run_bass_kernel_spmd(_NC_CACHE, in_maps, core_ids=list(range(NCORES)))
    y = res.results[0]["y"].astype(np.float32)
    for c in range(1, NCORES):
        y = y + res.results[c]["y"].astype(np.float32)
    return y.reshape(B, T, C)
